# revision 2
# baseline (speedup 1.0000x reference)
# Trainium2 (Bass/Tile) kernel for nn_AdaptiveFeaturePropagation.
#
# Sharding: pure data-parallel — batch B=8, one sample per NeuronCore, no
# collectives. Each core runs an identical graph; per-core in_maps carry that
# sample's tensors plus the (shared) rearranged weights.
#
# Per-core pipeline, streamed over row-chunks of R=8 image rows:
#   conv1 (x2 inputs), conv2, conv3 as tap-decomposed float32r matmuls
#   accumulated in PSUM; softmax is computed unnormalized as
#   e = max(exp(z+b3), 1) (= exp(relu(z+b3))) with the 1/sum(e) division
#   folded into the output evacuation; the per-pixel 7x7 dynamic conv runs
#   pixel-major on the Vector engine as fused (in0*scalar)+in1 ops
#   (scalar_tensor_tensor) in bf16, with the cross-partition W-shifts done by
#   the Tensor engine via shifted-identity matmuls accumulating fp32 in PSUM.
#
# Device-side layouts (per core):
#   xc, xk : [128, H+2, W+2] f32   host-padded cur_low / key_low (pad=1)
#   khT    : [H, W, 256] bf16      host-transposed key_high (pixel-major)
#   wred   : [128, 9, 256] f32     w_red as [ci, tap, co]
#   w2t    : [128, 9, 4, 256] f32  w2 as [ci_lo, tap, ci_tile, co]
#   w3t    : [128, 2, 49] f32      w3 as [ci_lo, ci_tile, q]
#   bred,b2: [128, 2] f32          biases as [co_lo, co_tile]
#   b3     : [49, 1] f32
#   identb : [128, 128] bf16       identity (transposes)
#   ishift : [128, 7, 128] bf16    ishift[v, jj, w] = 1 iff v == w + (jj-3)
#   out    : [H, W, 256] f32       pixel-major output (host transposes back)
from contextlib import ExitStack

import numpy as np
import ml_dtypes

import concourse.bass as bass
import concourse.mybir as mybir
import concourse.tile as tile
from concourse import bacc
from concourse.bass_utils import run_bass_kernel_spmd

F32 = mybir.dt.float32
F16 = mybir.dt.float16
BF16 = mybir.dt.bfloat16
AF = mybir.ActivationFunctionType
ALU = mybir.AluOpType
BF16_NP = ml_dtypes.bfloat16

KS = 7
CL = 128   # C_LOW
CH = 256   # C_HIGH
W = 128
B = 8
H_FULL = 128


def build_afp(tc, ins, outs, H=128, R=8):
    nc = tc.nc
    NCH = H // R
    assert H % R == 0 and R % 4 == 0

    with ExitStack() as ctx:
        const = ctx.enter_context(tc.tile_pool(name="const", bufs=1))
        p_in = ctx.enter_context(tc.tile_pool(name="p_in", bufs=2))
        p_ck = ctx.enter_context(tc.tile_pool(name="p_ck", bufs=2))
        p_x2 = ctx.enter_context(tc.tile_pool(name="p_x2", bufs=2))
        p_e2 = ctx.enter_context(tc.tile_pool(name="p_e2", bufs=2))
        p_fpt = ctx.enter_context(tc.tile_pool(name="p_fpt", bufs=2))
        p_kd = ctx.enter_context(tc.tile_pool(name="p_kd", bufs=3))
        p_rs = ctx.enter_context(tc.tile_pool(name="p_rs", bufs=3))
        p_g = ctx.enter_context(tc.tile_pool(name="p_g", bufs=8))
        p_out = ctx.enter_context(tc.tile_pool(name="p_out", bufs=2))
        ps_c = ctx.enter_context(tc.tile_pool(name="ps_c", bufs=3, space="PSUM"))
        ps_kd = ctx.enter_context(tc.tile_pool(name="ps_kd", bufs=2, space="PSUM"))
        ps_o = ctx.enter_context(tc.tile_pool(name="ps_o", bufs=2, space="PSUM"))

        # ---- constants ----
        wred = const.tile([128, 9, 256], F16, tag="wred")
        nc.sync.dma_start(wred[:], ins["wred"][:])
        w2 = const.tile([128, 9, 4, 256], F16, tag="w2t")
        nc.sync.dma_start(w2[:], ins["w2t"][:])
        w3 = const.tile([128, 2, 49], F16, tag="w3t")
        nc.sync.dma_start(w3[:], ins["w3t"][:])
        bred = const.tile([128, 2], F32, tag="bred")
        nc.sync.dma_start(bred[:], ins["bred"][:])
        b2 = const.tile([128, 2], F32, tag="b2")
        nc.sync.dma_start(b2[:], ins["b2"][:])
        b3 = const.tile([49, 1], F32, tag="b3")
        nc.sync.dma_start(b3[:], ins["b3"][:])
        identf = const.tile([128, 128], F32, tag="identf")
        nc.sync.dma_start(identf[:], ins["identf"][:])
        ishift = const.tile([128, 7, 128], BF16, tag="ishift")
        nc.sync.dma_start(ishift[:], ins["ishift"][:])

        for k in range(NCH):
            r0 = R * k                      # first conv2-output row of chunk
            c1_lo = max(0, r0 - 1)          # conv1 output rows [c1_lo, c1_hi)
            c1_hi = min(H, r0 + R + 1)
            base1 = r0 - 1                  # conv1-row -> cur/key slot: row - base1
            basei = r0 - 1                  # padded-input row -> xc slot
            in_lo, in_hi = c1_lo, c1_hi + 2

            # ---- input DMAs ----
            xc = p_in.tile([128, R + 4, W + 2], F16, tag="xc")
            nc.sync.dma_start(
                xc[:, in_lo - basei : in_hi - basei, :], ins["xc"][:, in_lo:in_hi, :]
            )
            xk = p_in.tile([128, R + 4, W + 2], F16, tag="xk")
            nc.sync.dma_start(
                xk[:, in_lo - basei : in_hi - basei, :], ins["xk"][:, in_lo:in_hi, :]
            )
            f_lo, f_hi = max(0, r0 - 3), min(H, r0 + R + 3)
            fpt = p_fpt.tile([128, R + 6, 256], BF16, tag="fpt")
            nc.sync.dma_start(
                fpt[:, f_lo - (r0 - 3) : f_hi - (r0 - 3), :],
                ins["khT"][f_lo:f_hi].rearrange("h w c -> w h c"),
            )

            # ---- conv1 (cur and key) ----
            cur = p_ck.tile([128, 2, R + 2, W + 2], F16, tag="cur")
            key = p_ck.tile([128, 2, R + 2, W + 2], F16, tag="key")
            for t in (cur, key):
                nc.vector.memset(t[:, :, :, 0:1], 0.0)
                nc.vector.memset(t[:, :, :, W + 1 : W + 2], 0.0)
                if k == 0:
                    nc.vector.memset(t[:, :, 0:1, :], 0.0)
                if k == NCH - 1:
                    nc.vector.memset(t[:, :, R + 1 : R + 2, :], 0.0)

            for src, dst in ((xc, cur), (xk, key)):
                g0 = c1_lo
                while g0 < c1_hi:
                    nr = min(4, c1_hi - g0)
                    for ct in range(2):
                        ps = ps_c.tile([128, 4, 128], F32, tag="cps")
                        for tap in range(9):
                            di, dj = tap // 3, tap % 3
                            nc.tensor.matmul(
                                ps[:, :nr, :],
                                wred[:, tap, ct * 128 : ct * 128 + 128],
                                src[
                                    :,
                                    g0 - basei + di : g0 - basei + di + nr,
                                    dj : dj + W,
                                ],
                                start=(tap == 0),
                                stop=(tap == 8),
                            )
                        nc.scalar.activation(
                            dst[:, ct, g0 - base1 : g0 - base1 + nr, 1 : W + 1],
                            ps[:, :nr, :],
                            AF.Relu,
                            bias=bred[:, ct : ct + 1],
                        )
                    g0 += nr

            # ---- conv2 ----
            x2 = p_x2.tile([128, 2, R, W], F16, tag="x2")
            for gg in range(R // 4):
                gr = r0 + gg * 4
                for ct in range(2):
                    ps = ps_c.tile([128, 4, 128], F32, tag="cps")
                    n_mm = 0
                    for s in range(4):
                        rhs_t = cur if s < 2 else key
                        for tap in range(9):
                            di, dj = tap // 3, tap % 3
                            sl = gr + di - 1 - base1
                            nc.tensor.matmul(
                                ps[:],
                                w2[:, tap, s, ct * 128 : ct * 128 + 128],
                                rhs_t[:, s % 2, sl : sl + 4, dj : dj + W],
                                start=(n_mm == 0),
                                stop=(n_mm == 35),
                            )
                            n_mm += 1
                    nc.scalar.activation(
                        x2[:, ct, gg * 4 : gg * 4 + 4, :],
                        ps[:],
                        AF.Relu,
                        bias=b2[:, ct : ct + 1],
                    )

            # ---- conv3 + exp ----
            e2 = p_e2.tile([49, R, W + 6], F32, tag="e2")
            nc.vector.memset(e2[:, :, 0:3], 0.0)
            nc.vector.memset(e2[:, :, W + 3 : W + 6], 0.0)
            for gg in range(R // 4):
                ps = ps_c.tile([49, 512], F32, tag="cps")
                for s in range(2):
                    nc.tensor.matmul(
                        ps[:],
                        w3[:, s, :],
                        x2[:, s, gg * 4 : gg * 4 + 4, :],
                        start=(s == 0),
                        stop=(s == 1),
                    )
                nc.scalar.activation(
                    e2[:, gg * 4 : gg * 4 + 4, 3 : W + 3],
                    ps[:],
                    AF.Exp,
                    bias=b3[:],
                )
            # e = exp(relu(z)) = max(exp(z), 1)
            nc.vector.tensor_scalar_max(
                e2[:, :, 3 : W + 3], e2[:, :, 3 : W + 3], 1.0
            )

            # ---- per-row: kernel maps + dynamic conv ----
            out_sb = p_out.tile([128, R, 256], F32, tag="osb")
            for lr in range(R):
                h = r0 + lr
                kdp = ps_kd.tile([128, 350], F32, tag="kdp")
                for jj in range(7):
                    dj = jj - 3
                    nc.tensor.transpose(
                        kdp[:, jj * 50 : jj * 50 + 49],
                        e2[:, lr, 3 - dj : 3 - dj + W],
                        identf[:49, :49],
                    )
                kd = p_kd.tile([128, 350], F32, tag="kd")
                nc.scalar.copy(
                    kd[:].rearrange("p (j q) -> p j q", q=50)[:, :, :49],
                    kdp[:].rearrange("p (j q) -> p j q", q=50)[:, :, :49],
                )
                # S = sum_q e[q, w] = row-sum of the unshifted (dj=0) block
                s_sum = p_rs.tile([128, 1], F32, tag="ssum")
                nc.vector.tensor_reduce(
                    s_sum[:], kd[:, 3 * 50 : 3 * 50 + 49], mybir.AxisListType.X, ALU.add
                )
                rs = p_rs.tile([128, 1], F32, tag="rs")
                nc.vector.reciprocal(rs[:], s_sum[:])

                op = ps_o.tile([128, 256], F32, tag="op")
                for jj in range(7):
                    dj = jj - 3
                    g = p_g.tile([128, 256], BF16, tag="g")
                    first = True
                    for ii in range(7):
                        di = ii - 3
                        t = h + di
                        if not (0 <= t < H):
                            continue
                        slot = t - (r0 - 3)
                        q = ii * 7 + jj
                        scal = kd[:, jj * 50 + q : jj * 50 + q + 1]
                        if first:
                            nc.vector.tensor_scalar_mul(
                                g[:], fpt[:, slot, :], scal
                            )
                            first = False
                        else:
                            nc.vector.scalar_tensor_tensor(
                                g[:], fpt[:, slot, :], scal, g[:],
                                ALU.mult, ALU.add,
                            )
                    nc.tensor.matmul(
                        op[:],
                        ishift[:, jj, :],
                        g[:],
                        start=(jj == 0),
                        stop=(jj == 6),
                    )
                nc.scalar.mul(out_sb[:, lr, :], op[:], mul=rs[:])

            nc.sync.dma_start(
                outs["out"][r0 : r0 + R].rearrange("h w c -> w h c"),
                out_sb[:],
            )


def host_prep_weights(w_red, b_red, w2, b2, w3, b3):
    """Host-side weight rearrangement. Returns dict of np arrays."""
    w_red = np.asarray(w_red, np.float32)
    w2 = np.asarray(w2, np.float32)
    w3 = np.asarray(w3, np.float32)
    wred_t = np.ascontiguousarray(
        w_red.reshape(256, 128, 9).transpose(1, 2, 0)
    )  # [ci, tap, co]
    w2_t = np.ascontiguousarray(
        w2.reshape(256, 4, 128, 9).transpose(2, 3, 1, 0)
    )  # [ci_lo, tap, s, co]
    w3_t = np.ascontiguousarray(
        w3.reshape(49, 2, 128).transpose(2, 1, 0)
    )  # [ci_lo, s, q]
    bred_t = np.ascontiguousarray(np.asarray(b_red, np.float32).reshape(2, 128).T)
    b2_t = np.ascontiguousarray(np.asarray(b2, np.float32).reshape(2, 128).T)
    b3_t = np.ascontiguousarray(np.asarray(b3, np.float32).reshape(49, 1))
    identf = np.eye(128, dtype=np.float32)
    ishift = np.ascontiguousarray(
        np.stack([np.eye(128, k=-(jj - 3), dtype=np.float32) for jj in range(7)], 0)
        .transpose(1, 0, 2)
        .astype(BF16_NP)
    )  # [v, jj, w]
    return {
        "wred": wred_t.astype(np.float16),
        "w2t": w2_t.astype(np.float16),
        "w3t": w3_t.astype(np.float16),
        "bred": bred_t,
        "b2": b2_t,
        "b3": b3_t,
        "identf": identf,
        "ishift": ishift,
    }


_IN_SPECS = [
    ("xc", [CL, H_FULL + 2, W + 2], F16),
    ("xk", [CL, H_FULL + 2, W + 2], F16),
    ("khT", [H_FULL, W, CH], BF16),
    ("wred", [128, 9, 256], F16),
    ("w2t", [128, 9, 4, 256], F16),
    ("w3t", [128, 2, 49], F16),
    ("bred", [128, 2], F32),
    ("b2", [128, 2], F32),
    ("b3", [49, 1], F32),
    ("identf", [128, 128], F32),
    ("ishift", [128, 7, 128], BF16),
]

_CACHE = {}


def _build_module(R=8):
    key = ("nc", R)
    if key in _CACHE:
        return _CACHE[key]
    nc = bacc.Bacc(
        "TRN2", target_bir_lowering=False, debug=False, num_devices=B
    )
    ins = {}
    for name, shape, dt in _IN_SPECS:
        ins[name] = nc.dram_tensor(name, shape, dt, kind="ExternalInput").ap()
    outs = {
        "out": nc.dram_tensor(
            "out", [H_FULL, W, CH], F32, kind="ExternalOutput"
        ).ap()
    }
    with tile.TileContext(nc) as tc:
        build_afp(tc, ins, outs, H=H_FULL, R=R)
    nc.compile()
    _CACHE[key] = nc
    return nc


def make_in_maps(cur_low, key_low, key_high, w_red, b_red, w2, b2, w3, b3):
    cur_low = np.asarray(cur_low, np.float32)
    key_low = np.asarray(key_low, np.float32)
    key_high = np.asarray(key_high, np.float32)
    wd = host_prep_weights(w_red, b_red, w2, b2, w3, b3)
    xc_all = np.pad(cur_low, ((0, 0), (0, 0), (1, 1), (1, 1))).astype(np.float16)
    xk_all = np.pad(key_low, ((0, 0), (0, 0), (1, 1), (1, 1))).astype(np.float16)
    khT_all = np.transpose(key_high, (0, 2, 3, 1)).astype(BF16_NP)
    in_maps = []
    for i in range(B):
        m = dict(wd)
        m["xc"] = np.ascontiguousarray(xc_all[i])
        m["xk"] = np.ascontiguousarray(xk_all[i])
        m["khT"] = np.ascontiguousarray(khT_all[i])
        in_maps.append(m)
    return in_maps


def run(inputs, trace=False, R=8):
    """Run on 8 NeuronCores; returns (out [8,256,128,128] f32, BassKernelResults)."""
    nc = _build_module(R=R)
    in_maps = make_in_maps(**inputs)
    res = run_bass_kernel_spmd(nc, in_maps, core_ids=list(range(B)), trace=trace)
    out = np.stack([res.results[i]["out"] for i in range(B)], axis=0)
    out = np.ascontiguousarray(out.transpose(0, 3, 1, 2))
    return out, res


def kernel(**inputs) -> np.ndarray:
    out, _ = run(inputs, trace=False)
    return out


# revision 3
# speedup vs baseline: 2.0389x; 2.0389x over previous
# Trainium2 (Bass/Tile) kernel for nn_AdaptiveFeaturePropagation.
#
# Sharding: pure data-parallel — batch B=8, one sample per NeuronCore, no
# collectives. Each core runs an identical graph; per-core in_maps carry that
# sample's tensors plus the (shared) rearranged weights.
#
# Per-core pipeline, streamed over row-chunks of R=8 image rows:
#   conv1 (x2 inputs), conv2, conv3 as tap-decomposed float32r matmuls
#   accumulated in PSUM; softmax is computed unnormalized as
#   e = max(exp(z+b3), 1) (= exp(relu(z+b3))) with the 1/sum(e) division
#   folded into the output evacuation; the per-pixel 7x7 dynamic conv runs
#   pixel-major on the Vector engine as fused (in0*scalar)+in1 ops
#   (scalar_tensor_tensor) in bf16, with the cross-partition W-shifts done by
#   the Tensor engine via shifted-identity matmuls accumulating fp32 in PSUM.
#
# Device-side layouts (per core):
#   xc, xk : [128, H+2, W+2] f32   host-padded cur_low / key_low (pad=1)
#   khT    : [H, W, 256] bf16      host-transposed key_high (pixel-major)
#   wred   : [128, 9, 256] f32     w_red as [ci, tap, co]
#   w2t    : [128, 9, 4, 256] f32  w2 as [ci_lo, tap, ci_tile, co]
#   w3t    : [128, 2, 49] f32      w3 as [ci_lo, ci_tile, q]
#   bred,b2: [128, 2] f32          biases as [co_lo, co_tile]
#   b3     : [49, 1] f32
#   identb : [128, 128] bf16       identity (transposes)
#   ishift : [128, 7, 128] bf16    ishift[v, jj, w] = 1 iff v == w + (jj-3)
#   out    : [H, W, 256] f32       pixel-major output (host transposes back)
from contextlib import ExitStack

import numpy as np
import ml_dtypes

import concourse.bass as bass
import concourse.mybir as mybir
import concourse.tile as tile
from concourse import bacc
from concourse.bass_utils import run_bass_kernel_spmd

F32 = mybir.dt.float32
F16 = mybir.dt.float16
BF16 = mybir.dt.bfloat16
AF = mybir.ActivationFunctionType
ALU = mybir.AluOpType
BF16_NP = ml_dtypes.bfloat16

KS = 7
CL = 128   # C_LOW
CH = 256   # C_HIGH
W = 128
B = 8
H_FULL = 128


def build_afp(tc, ins, outs, H=128, R=8, nrep=1):
    nc = tc.nc
    NCH = H // R
    assert H % R == 0 and R % 4 == 0

    with ExitStack() as ctx:
        const = ctx.enter_context(tc.tile_pool(name="const", bufs=1))
        p_in = ctx.enter_context(tc.tile_pool(name="p_in", bufs=2))
        p_ck = ctx.enter_context(tc.tile_pool(name="p_ck", bufs=2))
        p_x2 = ctx.enter_context(tc.tile_pool(name="p_x2", bufs=2))
        p_e2 = ctx.enter_context(tc.tile_pool(name="p_e2", bufs=2))
        p_fpt = ctx.enter_context(tc.tile_pool(name="p_fpt", bufs=2))
        p_kd = ctx.enter_context(tc.tile_pool(name="p_kd", bufs=3))
        p_rs = ctx.enter_context(tc.tile_pool(name="p_rs", bufs=3))
        p_g = ctx.enter_context(tc.tile_pool(name="p_g", bufs=8))
        p_out = ctx.enter_context(tc.tile_pool(name="p_out", bufs=2))
        ps_c = ctx.enter_context(tc.tile_pool(name="ps_c", bufs=3, space="PSUM"))
        ps_kd = ctx.enter_context(tc.tile_pool(name="ps_kd", bufs=2, space="PSUM"))
        ps_o = ctx.enter_context(tc.tile_pool(name="ps_o", bufs=2, space="PSUM"))

        # ---- constants ----
        wred = const.tile([128, 9, 256], F16, tag="wred")
        nc.sync.dma_start(wred[:], ins["wred"][:])
        w2 = const.tile([128, 9, 4, 256], F16, tag="w2t")
        nc.sync.dma_start(w2[:], ins["w2t"][:])
        w3 = const.tile([128, 2, 49], F16, tag="w3t")
        nc.sync.dma_start(w3[:], ins["w3t"][:])
        bred = const.tile([128, 2], F32, tag="bred")
        nc.sync.dma_start(bred[:], ins["bred"][:])
        b2 = const.tile([128, 2], F32, tag="b2")
        nc.sync.dma_start(b2[:], ins["b2"][:])
        b3 = const.tile([49, 1], F32, tag="b3")
        nc.sync.dma_start(b3[:], ins["b3"][:])
        identf = const.tile([128, 128], F32, tag="identf")
        nc.sync.dma_start(identf[:], ins["identf"][:])
        ishift = const.tile([128, 7, 128], BF16, tag="ishift")
        nc.sync.dma_start(ishift[:], ins["ishift"][:])

        for k in [kk for _ in range(nrep) for kk in range(NCH)]:
            r0 = R * k                      # first conv2-output row of chunk
            c1_lo = max(0, r0 - 1)          # conv1 output rows [c1_lo, c1_hi)
            c1_hi = min(H, r0 + R + 1)
            base1 = r0 - 1                  # conv1-row -> cur/key slot: row - base1
            basei = r0 - 1                  # padded-input row -> xc slot
            in_lo, in_hi = c1_lo, c1_hi + 2

            # ---- input DMAs ----
            xc = p_in.tile([128, R + 4, W + 2], F16, tag="xc")
            nc.sync.dma_start(
                xc[:, in_lo - basei : in_hi - basei, :], ins["xc"][:, in_lo:in_hi, :]
            )
            xk = p_in.tile([128, R + 4, W + 2], F16, tag="xk")
            nc.sync.dma_start(
                xk[:, in_lo - basei : in_hi - basei, :], ins["xk"][:, in_lo:in_hi, :]
            )
            f_lo, f_hi = max(0, r0 - 3), min(H, r0 + R + 3)
            fpt = p_fpt.tile([128, R + 6, 256], BF16, tag="fpt")
            nc.sync.dma_start(
                fpt[:, f_lo - (r0 - 3) : f_hi - (r0 - 3), :],
                ins["khT"][f_lo:f_hi].rearrange("h w c -> w h c"),
            )

            # ---- conv1 (cur and key) ----
            cur = p_ck.tile([128, 2, R + 2, W + 2], F16, tag="cur")
            key = p_ck.tile([128, 2, R + 2, W + 2], F16, tag="key")
            for t in (cur, key):
                nc.vector.memset(t[:, :, :, 0:1], 0.0)
                nc.vector.memset(t[:, :, :, W + 1 : W + 2], 0.0)
                if k == 0:
                    nc.vector.memset(t[:, :, 0:1, :], 0.0)
                if k == NCH - 1:
                    nc.vector.memset(t[:, :, R + 1 : R + 2, :], 0.0)

            for src, dst in ((xc, cur), (xk, key)):
                g0 = c1_lo
                while g0 < c1_hi:
                    nr = min(4, c1_hi - g0)
                    for ct in range(2):
                        ps = ps_c.tile([128, 4, 128], F32, tag="cps")
                        for tap in range(9):
                            di, dj = tap // 3, tap % 3
                            nc.tensor.matmul(
                                ps[:, :nr, :],
                                wred[:, tap, ct * 128 : ct * 128 + 128],
                                src[
                                    :,
                                    g0 - basei + di : g0 - basei + di + nr,
                                    dj : dj + W,
                                ],
                                start=(tap == 0),
                                stop=(tap == 8),
                            )
                        nc.scalar.activation(
                            dst[:, ct, g0 - base1 : g0 - base1 + nr, 1 : W + 1],
                            ps[:, :nr, :],
                            AF.Relu,
                            bias=bred[:, ct : ct + 1],
                        )
                    g0 += nr

            # ---- conv2 ----
            x2 = p_x2.tile([128, 2, R, W], F16, tag="x2")
            for gg in range(R // 4):
                gr = r0 + gg * 4
                for ct in range(2):
                    ps = ps_c.tile([128, 4, 128], F32, tag="cps")
                    n_mm = 0
                    for s in range(4):
                        rhs_t = cur if s < 2 else key
                        for tap in range(9):
                            di, dj = tap // 3, tap % 3
                            sl = gr + di - 1 - base1
                            nc.tensor.matmul(
                                ps[:],
                                w2[:, tap, s, ct * 128 : ct * 128 + 128],
                                rhs_t[:, s % 2, sl : sl + 4, dj : dj + W],
                                start=(n_mm == 0),
                                stop=(n_mm == 35),
                            )
                            n_mm += 1
                    nc.scalar.activation(
                        x2[:, ct, gg * 4 : gg * 4 + 4, :],
                        ps[:],
                        AF.Relu,
                        bias=b2[:, ct : ct + 1],
                    )

            # ---- conv3 + exp ----
            e2 = p_e2.tile([49, R, W + 6], F32, tag="e2")
            nc.vector.memset(e2[:, :, 0:3], 0.0)
            nc.vector.memset(e2[:, :, W + 3 : W + 6], 0.0)
            for gg in range(R // 4):
                ps = ps_c.tile([49, 512], F32, tag="cps")
                for s in range(2):
                    nc.tensor.matmul(
                        ps[:],
                        w3[:, s, :],
                        x2[:, s, gg * 4 : gg * 4 + 4, :],
                        start=(s == 0),
                        stop=(s == 1),
                    )
                nc.scalar.activation(
                    e2[:, gg * 4 : gg * 4 + 4, 3 : W + 3],
                    ps[:],
                    AF.Exp,
                    bias=b3[:],
                )
            # e = exp(relu(z)) = max(exp(z), 1)
            nc.vector.tensor_scalar_max(
                e2[:, :, 3 : W + 3], e2[:, :, 3 : W + 3], 1.0
            )

            # ---- per-row: kernel maps + dynamic conv ----
            out_sb = p_out.tile([128, R, 256], F32, tag="osb")
            for lr in range(R):
                h = r0 + lr
                kdp = ps_kd.tile([128, 350], F32, tag="kdp")
                for jj in range(7):
                    dj = jj - 3
                    nc.tensor.transpose(
                        kdp[:, jj * 50 : jj * 50 + 49],
                        e2[:, lr, 3 - dj : 3 - dj + W],
                        identf[:49, :49],
                    )
                kd = p_kd.tile([128, 350], F32, tag="kd")
                nc.scalar.copy(
                    kd[:].rearrange("p (j q) -> p j q", q=50)[:, :, :49],
                    kdp[:].rearrange("p (j q) -> p j q", q=50)[:, :, :49],
                )
                # S = sum_q e[q, w] = row-sum of the unshifted (dj=0) block
                s_sum = p_rs.tile([128, 1], F32, tag="ssum")
                nc.vector.tensor_reduce(
                    s_sum[:], kd[:, 3 * 50 : 3 * 50 + 49], mybir.AxisListType.X, ALU.add
                )
                rs = p_rs.tile([128, 1], F32, tag="rs")
                nc.vector.reciprocal(rs[:], s_sum[:])

                op = ps_o.tile([128, 256], F32, tag="op")
                for jj in range(7):
                    dj = jj - 3
                    g = p_g.tile([128, 256], BF16, tag="g")
                    first = True
                    for ii in range(7):
                        di = ii - 3
                        t = h + di
                        if not (0 <= t < H):
                            continue
                        slot = t - (r0 - 3)
                        q = ii * 7 + jj
                        scal = kd[:, jj * 50 + q : jj * 50 + q + 1]
                        if first:
                            nc.vector.tensor_scalar_mul(
                                g[:], fpt[:, slot, :], scal
                            )
                            first = False
                        else:
                            nc.vector.scalar_tensor_tensor(
                                g[:], fpt[:, slot, :], scal, g[:],
                                ALU.mult, ALU.add,
                            )
                    nc.tensor.matmul(
                        op[:],
                        ishift[:, jj, :],
                        g[:],
                        start=(jj == 0),
                        stop=(jj == 6),
                    )
                nc.scalar.mul(out_sb[:, lr, :], op[:], mul=rs[:])

            nc.sync.dma_start(
                outs["out"][r0 : r0 + R].rearrange("h w c -> w h c"),
                out_sb[:],
            )


def host_prep_weights(w_red, b_red, w2, b2, w3, b3):
    """Host-side weight rearrangement. Returns dict of np arrays."""
    w_red = np.asarray(w_red, np.float32)
    w2 = np.asarray(w2, np.float32)
    w3 = np.asarray(w3, np.float32)
    wred_t = np.ascontiguousarray(
        w_red.reshape(256, 128, 9).transpose(1, 2, 0)
    )  # [ci, tap, co]
    w2_t = np.ascontiguousarray(
        w2.reshape(256, 4, 128, 9).transpose(2, 3, 1, 0)
    )  # [ci_lo, tap, s, co]
    w3_t = np.ascontiguousarray(
        w3.reshape(49, 2, 128).transpose(2, 1, 0)
    )  # [ci_lo, s, q]
    bred_t = np.ascontiguousarray(np.asarray(b_red, np.float32).reshape(2, 128).T)
    b2_t = np.ascontiguousarray(np.asarray(b2, np.float32).reshape(2, 128).T)
    b3_t = np.ascontiguousarray(np.asarray(b3, np.float32).reshape(49, 1))
    identf = np.eye(128, dtype=np.float32)
    ishift = np.ascontiguousarray(
        np.stack([np.eye(128, k=-(jj - 3), dtype=np.float32) for jj in range(7)], 0)
        .transpose(1, 0, 2)
        .astype(BF16_NP)
    )  # [v, jj, w]
    return {
        "wred": wred_t.astype(np.float16),
        "w2t": w2_t.astype(np.float16),
        "w3t": w3_t.astype(np.float16),
        "bred": bred_t,
        "b2": b2_t,
        "b3": b3_t,
        "identf": identf,
        "ishift": ishift,
    }


_IN_SPECS = [
    ("xc", [CL, H_FULL + 2, W + 2], F16),
    ("xk", [CL, H_FULL + 2, W + 2], F16),
    ("khT", [H_FULL, W, CH], BF16),
    ("wred", [128, 9, 256], F16),
    ("w2t", [128, 9, 4, 256], F16),
    ("w3t", [128, 2, 49], F16),
    ("bred", [128, 2], F32),
    ("b2", [128, 2], F32),
    ("b3", [49, 1], F32),
    ("identf", [128, 128], F32),
    ("ishift", [128, 7, 128], BF16),
]

_CACHE = {}


def _build_module(R=8, nrep=1):
    key = ("nc", R, nrep)
    if key in _CACHE:
        return _CACHE[key]
    nc = bacc.Bacc(
        "TRN2", target_bir_lowering=False, debug=False, num_devices=B
    )
    ins = {}
    for name, shape, dt in _IN_SPECS:
        ins[name] = nc.dram_tensor(name, shape, dt, kind="ExternalInput").ap()
    outs = {
        "out": nc.dram_tensor(
            "out", [H_FULL, W, CH], F32, kind="ExternalOutput"
        ).ap()
    }
    with tile.TileContext(nc) as tc:
        build_afp(tc, ins, outs, H=H_FULL, R=R, nrep=nrep)
    nc.compile()
    _CACHE[key] = nc
    return nc


def make_in_maps(cur_low, key_low, key_high, w_red, b_red, w2, b2, w3, b3):
    cur_low = np.asarray(cur_low, np.float32)
    key_low = np.asarray(key_low, np.float32)
    key_high = np.asarray(key_high, np.float32)
    wd = host_prep_weights(w_red, b_red, w2, b2, w3, b3)
    xc_all = np.pad(cur_low, ((0, 0), (0, 0), (1, 1), (1, 1))).astype(np.float16)
    xk_all = np.pad(key_low, ((0, 0), (0, 0), (1, 1), (1, 1))).astype(np.float16)
    khT_all = np.transpose(key_high, (0, 2, 3, 1)).astype(BF16_NP)
    in_maps = []
    for i in range(B):
        m = dict(wd)
        m["xc"] = np.ascontiguousarray(xc_all[i])
        m["xk"] = np.ascontiguousarray(xk_all[i])
        m["khT"] = np.ascontiguousarray(khT_all[i])
        in_maps.append(m)
    return in_maps


def run(inputs, trace=False, R=8):
    """Run on 8 NeuronCores; returns (out [8,256,128,128] f32, BassKernelResults)."""
    nc = _build_module(R=R)
    in_maps = make_in_maps(**inputs)
    res = run_bass_kernel_spmd(nc, in_maps, core_ids=list(range(B)), trace=trace)
    out = np.stack([res.results[i]["out"] for i in range(B)], axis=0)
    out = np.ascontiguousarray(out.transpose(0, 3, 1, 2))
    return out, res


def kernel(**inputs) -> np.ndarray:
    out, _ = run(inputs, trace=False)
    return out
